# revision 1
# baseline (speedup 1.0000x reference)
"""Trainium2 Bass kernel for a single causal self-attention head.

Reference computation (fp32):
    Q = q @ Wq; K = q @ Wk; V = q @ Wv          # q: [B, T, D]
    scores = Q K^T / sqrt(D)  (causal masked)
    out = softmax(scores) @ V                    # [B, T, dv]

Shapes are hardcoded: B=512, T=200, D=1024, dk=dv=64, 8 NeuronCores,
batch-sharded 64 per core (pure data parallel, weights replicated).

Per-core dataflow (64 batches processed as 32 pairs, software-pipelined:
attention for pair p-1 is emitted between projection rounds so the PE
never stalls on the PSUM-evacuation / exp chain):
  - host feeds qT laid out [b, 128, 8*200] so every DMA run is a full
    contiguous 6.4KB per partition (128 descriptors per batch load)
  - projections: stationary [Wq|Wk] (and Wv) per 128-row d-tile, moving
    operand is qT for a PAIR of batches (N=400) in float32r (full-rate
    fp32 with 11-bit mantissa; host pre-rounds so results are exact)
    accumulating over 8 d-tiles -> PSUM holds [Q^T; K^T] stacked and V^T
  - scores^T = K Q^T per s-tile (E=[s,t] layout so softmax sums become a
    ones-column matmul); exp on ScalarE (no max subtraction needed:
    |scores| is O(1) by construction), causal mask via precomputed 0/1
    mask multiply
  - V^T transposed back to [s, v] on the PE, ones column appended; then
    U = E^T @ [V|1] accumulates both the numerator and the softmax
    denominator; final normalize is fused into the PSUM->SBUF copy via a
    per-partition reciprocal scale on ScalarE.
"""

import numpy as np

import concourse.bass as bass
import concourse.tile as tile
from concourse import bacc, mybir
from concourse.bass_utils import run_bass_kernel_spmd

B, T, D = 512, 200, 1024
DK = 64
N_CORES = 8
B_CORE = B // N_CORES  # 64
ND = D // 128  # 8 d-tiles
F32 = mybir.dt.float32
F32R = mybir.dt.float32r

# [(start, width)] tiles of the T=200 axis on 128 partitions (s axis)
T_TILES = [(0, 128), (128, 72)]
# Output rows are computed in two interleaved tiles (even t, odd t) so both
# pack into one SBUF tile whose per-partition 512B run is contiguous in HBM.


def build_nc(n_batch=B_CORE, use_f32r=True, repeat=1):
    """Build the per-core Bass module. Same program on all cores (SPMD)."""
    nc = bacc.Bacc("TRN2")

    # qT is host-prepped as [b, 128, ND*T]: partition p holds d-tiles
    # d*128+p, all contiguous per partition for 1-descriptor-per-partition
    # DMAs.
    qT = nc.dram_tensor("qT", [n_batch, 128, ND * T], F32, kind="ExternalInput")
    wqk = nc.dram_tensor("wqk", [D, 128], F32, kind="ExternalInput")
    wv = nc.dram_tensor("wv", [D, DK], F32, kind="ExternalInput")
    mask0 = nc.dram_tensor("mask0", [128, T], F32, kind="ExternalInput")
    mask1 = nc.dram_tensor("mask1", [72, T], F32, kind="ExternalInput")
    ident = nc.dram_tensor("ident", [128, 128], F32, kind="ExternalInput")
    shmat = nc.dram_tensor("shmat", [128, 64], F32, kind="ExternalInput")
    out = nc.dram_tensor("out", [n_batch, T, DK], F32, kind="ExternalOutput")

    assert n_batch % 2 == 0
    n_pair = n_batch // 2
    mmdt = F32R if use_f32r else F32

    with tile.TileContext(nc) as tc:
        with (
            tc.tile_pool(name="singles", bufs=1) as singles,
            tc.tile_pool(name="qt", bufs=4) as qt_pool,
            tc.tile_pool(name="sb", bufs=3) as sb_pool,
            tc.tile_pool(name="esb", bufs=4) as esb_pool,
            tc.tile_pool(name="vsb", bufs=4) as vsb_pool,
            tc.tile_pool(name="osb", bufs=12) as osb_pool,
            tc.tile_pool(name="ps_proj", bufs=1, space="PSUM") as ps_proj,
            tc.tile_pool(name="ps_vtr", bufs=1, space="PSUM") as ps_vtr,
            tc.tile_pool(name="ps_kt", bufs=1, space="PSUM") as ps_kt,
            tc.tile_pool(name="ps_e", bufs=2, space="PSUM") as ps_e,
            tc.tile_pool(name="ps_u", bufs=2, space="PSUM") as ps_u,
        ):
            # ---- constants, loaded once ----
            wqk_sb = singles.tile([128, ND, 128], mmdt)
            nc.sync.dma_start(
                out=wqk_sb, in_=wqk.rearrange("(d p) j -> p d j", p=128).bitcast(mmdt)
            )
            wv_sb = singles.tile([128, ND, DK], mmdt)
            nc.sync.dma_start(
                out=wv_sb, in_=wv.rearrange("(d p) j -> p d j", p=128).bitcast(mmdt)
            )
            m0_sb = singles.tile([128, T], F32)
            nc.sync.dma_start(out=m0_sb, in_=mask0[:, :])
            m1_sb = singles.tile([72, T], F32)
            nc.sync.dma_start(out=m1_sb, in_=mask1[:, :])
            id_sb = singles.tile([128, 128], F32)
            nc.sync.dma_start(out=id_sb, in_=ident[:, :])
            sh_sb = singles.tile([128, 64], F32R)
            nc.sync.dma_start(out=sh_sb, in_=shmat[:, :].bitcast(F32R))

            def emit_projection(p):
                """Load qT for pair p, project, evacuate. Returns tiles the
                attention stage needs."""
                # [pp, b, d*t]: per batch a fully contiguous 6.4KB/partition
                qt = qt_pool.tile([128, 2, ND * T], mmdt)
                Hs = 5 * T  # gpsimd-issued share (5 of 8 d-tiles)
                for bi in range(2):
                    nc.gpsimd.dma_start(
                        out=qt[:, bi, 0:Hs],
                        in_=qT[2 * p + bi][:, 0:Hs].bitcast(mmdt),
                    )
                    nc.sync.dma_start(
                        out=qt[:, bi, Hs : ND * T],
                        in_=qT[2 * p + bi][:, Hs : ND * T].bitcast(mmdt),
                    )

                # qk_ps rows 0:64 = Q^T, rows 64:128 = K^T (both batches)
                qk_ps = ps_proj.tile([128, 2 * T], F32)
                vt_ps = ps_proj.tile([64, 2 * T], F32, tag="vt_ps")
                qt4 = qt.rearrange("pp b (d t) -> pp b d t", d=ND)
                for d in range(ND):
                    rhs = qt4[:, :, d, :]  # [128, 2, 200] strided
                    nc.tensor.matmul(
                        qk_ps, wqk_sb[:, d, :], rhs,
                        start=(d == 0), stop=(d == ND - 1),
                    )
                    nc.tensor.matmul(
                        vt_ps, wv_sb[:, d, :], rhs,
                        start=(d == 0), stop=(d == ND - 1),
                    )

                # evacuate PSUM; shift K^T down to partition base 0
                qk_sb = sb_pool.tile([128, 2 * T], F32R)
                nc.scalar.copy(qk_sb, qk_ps)
                vt_sb = sb_pool.tile([64, 2 * T], F32, tag="vt_sb")
                nc.vector.tensor_copy(vt_sb[:, 0:T], vt_ps[:, 0:T])
                nc.vector.tensor_copy(vt_sb[:, T : 2 * T], vt_ps[:, T : 2 * T])
                ks_ps = ps_kt.tile([64, 2 * T], F32, tag="ks_ps")
                nc.tensor.matmul(ks_ps, sh_sb, qk_sb, start=True, stop=True)
                kt_sb = sb_pool.tile([64, 2 * T], F32R, tag="kt_sb")
                nc.scalar.copy(kt_sb[:, 0:T], ks_ps[:, 0:T])
                nc.scalar.copy(kt_sb[:, T : 2 * T], ks_ps[:, T : 2 * T])
                return qk_sb, vt_sb, kt_sb

            def emit_attention(p, qk_sb, vt_sb, kt_sb):
                for bi in range(2):
                    b = 2 * p + bi
                    c0 = bi * T  # column offset of this batch in pair tiles

                    # V^T -> V (plus ones column) via PE transpose
                    vtr = ps_vtr.tile([128, 128], F32)
                    nc.tensor.transpose(
                        vtr[:, 0:64],
                        vt_sb[0:64, c0 : c0 + 128],
                        id_sb[0:64, 0:64],
                    )
                    nc.tensor.transpose(
                        vtr[0:72, 64:128],
                        vt_sb[0:64, c0 + 128 : c0 + 200],
                        id_sb[0:64, 0:64],
                    )
                    v0 = vsb_pool.tile([128, 65], F32, tag="v0")
                    nc.vector.tensor_copy(v0[:, 0:64], vtr[:, 0:64])
                    nc.vector.memset(v0[:, 64:65], 1.0)
                    v1 = vsb_pool.tile([72, 65], F32, tag="v1")
                    nc.vector.tensor_copy(v1[:, 0:64], vtr[0:72, 64:128])
                    nc.vector.memset(v1[:, 64:65], 1.0)
                    vaug = [v0, v1]

                    # scores^T (E) per s-tile: exp(K Q^T / 32) * causal mask.
                    # The matmul streams a 256-wide window of the pair-wide
                    # Q^T covering this batch's 200 columns: N>=256 keeps
                    # fp32r at 1 cycle/row with minimal wasted columns.
                    w0 = 0 if bi == 0 else 2 * T - 256
                    e_tiles = []
                    for si, (s0, sw) in enumerate(T_TILES):
                        e_ps = ps_e.tile([sw, 256], F32, tag="e_ps")
                        nc.tensor.matmul(
                            e_ps,
                            kt_sb[0:64, c0 + s0 : c0 + s0 + sw],
                            qk_sb[0:64, w0 : w0 + 256],
                            start=True,
                            stop=True,
                        )
                        e_sb = esb_pool.tile([sw, T], F32, tag="e_sb")
                        nc.scalar.activation(
                            e_sb, e_ps[:, c0 - w0 : c0 - w0 + T],
                            mybir.ActivationFunctionType.Exp,
                            scale=1.0 / 32.0,
                        )
                        msk = m0_sb if si == 0 else m1_sb
                        nc.vector.tensor_mul(e_sb, e_sb, msk)
                        e_tiles.append(e_sb)

                    # U = E^T @ [V | 1]; normalize; store.
                    # Two interleaved 100-row t-tiles (even/odd) pack into one
                    # o_sb: partition p holds rows 2p and 2p+1 -> one DMA per
                    # batch with 512B contiguous runs.
                    o_sb = osb_pool.tile([100, 2, DK], F32, tag="o_sb")
                    e_pairs = [
                        e.rearrange("s (t c) -> s t c", c=2) for e in e_tiles
                    ]
                    for ci in range(2):
                        u_ps = ps_u.tile([100, 65], F32, tag="u_ps")
                        for si, (s0, sw) in enumerate(T_TILES):
                            nc.tensor.matmul(
                                u_ps,
                                e_pairs[si][:, :, ci],
                                vaug[si],
                                start=(si == 0),
                                stop=(si == 1),
                            )
                        r_sb = osb_pool.tile([100, 1], F32, tag="r_sb")
                        nc.vector.reciprocal(r_sb, u_ps[:, 64:65])
                        nc.scalar.activation(
                            o_sb[:, ci, :], u_ps[:, 0:64],
                            mybir.ActivationFunctionType.Copy, scale=r_sb,
                        )
                    nc.sync.dma_start(
                        out=out[b].rearrange("(t c) v -> t (c v)", c=2), in_=o_sb
                    )

            # software pipeline: projections run one pair ahead of attention
            for _rep in range(repeat):
                prev = None
                for p in range(n_pair):
                    tiles = emit_projection(p)
                    if prev is not None:
                        emit_attention(p - 1, *prev)
                    prev = tiles
                emit_attention(n_pair - 1, *prev)

    nc.compile()
    return nc


def round_f32r(a):
    """Round fp32 to the PE's fp32r format (11-bit mantissa, RNE)."""
    b = np.ascontiguousarray(a, dtype=np.float32).view(np.uint32)
    r = (b + 0x7FF + ((b >> 12) & 1)) & np.uint32(0xFFFFF000)
    return r.astype(np.uint32).view(np.float32)


def _host_inputs(q, Wq, Wk, Wv, use_f32r=True):
    """Shared (replicated) device inputs + per-core qT shards."""
    wqk = np.ascontiguousarray(np.concatenate([Wq, Wk], axis=1), dtype=np.float32)
    wv = np.ascontiguousarray(Wv, dtype=np.float32)
    if use_f32r:
        wqk, wv = round_f32r(wqk), round_f32r(wv)
    t_idx = np.arange(T)[None, :]
    m0 = (t_idx >= np.arange(128)[:, None]).astype(np.float32)
    m1 = (t_idx >= (128 + np.arange(72))[:, None]).astype(np.float32)
    ident = np.eye(128, dtype=np.float32)
    shmat = np.zeros((128, 64), dtype=np.float32)
    shmat[np.arange(64) + 64, np.arange(64)] = 1.0
    # [B, T, D] -> [B, D, T] -> [B, ND, 128, T] -> [B, 128, ND, T]:
    # partition p holds rows d*128+p of q^T, contiguous per partition.
    nb = q.shape[0]
    qT = np.ascontiguousarray(
        q.transpose(0, 2, 1)
        .reshape(nb, ND, 128, T)
        .transpose(0, 2, 1, 3)
        .reshape(nb, 128, ND * T)
    )
    if use_f32r:
        qT = round_f32r(qT)
    return qT, {
        "wqk": wqk, "wv": wv, "mask0": m0, "mask1": m1,
        "ident": ident, "shmat": shmat,
    }


_NC_CACHE = {}


def _get_nc(n_batch=B_CORE, use_f32r=True, repeat=1):
    key = (n_batch, use_f32r, repeat)
    if key not in _NC_CACHE:
        _NC_CACHE[key] = build_nc(n_batch, use_f32r, repeat)
    return _NC_CACHE[key]


def kernel(q, Wq, Wk, Wv):
    q = np.asarray(q, dtype=np.float32)
    qT, shared = _host_inputs(q, np.asarray(Wq), np.asarray(Wk), np.asarray(Wv))

    nc = _get_nc()
    in_maps = [
        {"qT": np.ascontiguousarray(qT[c * B_CORE : (c + 1) * B_CORE]), **shared}
        for c in range(N_CORES)
    ]
    res = run_bass_kernel_spmd(nc, in_maps, core_ids=list(range(N_CORES)))
    return np.concatenate([r["out"] for r in res.results], axis=0)



# revision 8
# speedup vs baseline: 1.1977x; 1.1977x over previous
"""Trainium2 Bass kernel for a single causal self-attention head.

Reference computation (fp32):
    Q = q @ Wq; K = q @ Wk; V = q @ Wv          # q: [B, T, D]
    scores = Q K^T / sqrt(D)  (causal masked)
    out = softmax(scores) @ V                    # [B, T, dv]

Shapes hardcoded: B=512, T=200, D=1024, dk=dv=64, 8 NeuronCores,
batch-sharded 64 per core (pure data parallel, weights replicated).

Key facts this implementation is built around (TRN2 cost model):
  - DMA transfers serialize on one shared 16-engine pool (~360 B/ns per
    core), so total HBM bytes is the wall: q is shipped as fp8 hi+lo
    (q8 = e4m3(q), r8 = e4m3(q-q8)) -> 2 B/elem instead of 4.
  - fp8e4 matmuls in DoubleRow perf mode process 2 k-tiles per
    instruction at 0.5 cycles/row -> 4x f32r projection throughput.
  - bf16 matmuls run 1 cycle/row at any width (f32r needs >=256 cols,
    fp32 costs 4x), so all attention-stage matmuls run in bf16.
  - GPSIMD (Pool) cannot touch PSUM, so it handles the SBUF-only mask
    multiplies and memsets while ACT/DVE split exp + PSUM evacuations.

Per-core dataflow (64 batches as 32 pairs, software-pipelined so the
PE, the DMA pool, and the ACT/DVE/Pool engines all stay busy):
  - host ships, per pair, one 6400B/partition fp8 block holding q8 and
    r8 in DoubleRow layout [128p][hi/lo][4 kpair][2 ksub][2 batch][200]
  - projections (DoubleRow fp8): Q^T and K^T each accumulate into their
    own [64, 400] PSUM tile (no K partition-shift needed); V uses three
    passes (q8@Wv8 + r8@Wv8 into rows 0:64, q8@rho8 into rows 64:128,
    where rho8 = e4m3(32Wv - Wv8) is the weight-quantization residual)
  - weights are pre-scaled by 32 on host so fp8 sees ~N(0,1) values;
    the exp activation scale (2^-15) and a 32-valued ones column absorb
    the scaling exactly
  - V^T halves are folded and transposed in one bf16 matmul against a
    stacked-identity [I;I] moving operand
  - scores^T = K Q^T in bf16, s-tile 2 (s>=128) only computes its valid
    t>=128 window; exp on ACT (psum -> bf16, scale 2^-15), causal mask
    via bf16 multiply on GPSIMD
  - U = E^T @ [V | 32]: per even/odd-t interleave, the s-tile-2 matmul
    lands on PSUM partitions 64:100 only; the [numerator | denominator]
    PSUM tile is DMA'd straight to HBM and the final divide runs on the
    host (softmax normalization is scale-exact, so this costs nothing)
"""

import numpy as np
import ml_dtypes

import concourse.bass as bass
import concourse.tile as tile
from concourse import bacc, mybir
from concourse.bass_utils import run_bass_kernel_spmd

B, T, D = 512, 200, 1024
DK = 64
N_CORES = 8
B_CORE = B // N_CORES  # 64
NKP = D // 256  # 4 DoubleRow k-tile pairs
F32 = mybir.dt.float32
BF16 = mybir.dt.bfloat16
F8 = mybir.dt.float8e4
DR = mybir.MatmulPerfMode.DoubleRow
E4M3 = ml_dtypes.float8_e4m3
NPBF16 = ml_dtypes.bfloat16

# vu PSUM tile layout (f32 element column offsets within one bank):
# fold outputs (V rows 0:128 / 128:200) and the s-tile-2 scores block
VTR0, VTR1, E2_COL, VU_COLS = 0, 64, 128, 200


def build_nc(n_batch=B_CORE, repeat=1):
    """Build the per-core Bass module. Same program on all cores (SPMD)."""
    nc = bacc.Bacc("TRN2")
    assert n_batch % 2 == 0
    n_pair = n_batch // 2

    qr = nc.dram_tensor("qr", [n_pair, 128, 2 * NKP * 2 * 2 * T], F8,
                        kind="ExternalInput")
    wq = nc.dram_tensor("wq", [128, NKP * 2 * DK], F8, kind="ExternalInput")
    wk = nc.dram_tensor("wk", [128, NKP * 2 * DK], F8, kind="ExternalInput")
    wv = nc.dram_tensor("wv", [128, NKP * 2 * DK], F8, kind="ExternalInput")
    wr = nc.dram_tensor("wr", [128, NKP * 2 * DK], F8, kind="ExternalInput")
    m1 = nc.dram_tensor("m1", [128, T], BF16, kind="ExternalInput")
    m2 = nc.dram_tensor("m2", [72, 72], BF16, kind="ExternalInput")
    fold = nc.dram_tensor("fold", [64, DK], BF16, kind="ExternalInput")
    # [num | den] staging: per pair [100 t-partitions, 2 batches, 2*65]
    uo = nc.dram_tensor("uo", [n_pair, 100, 2 * 2 * 65], F32,
                        kind="ExternalOutput")

    with tile.TileContext(nc) as tc:
        with (
            tc.tile_pool(name="singles", bufs=1) as singles,
            tc.tile_pool(name="qr", bufs=4) as qr_pool,
            tc.tile_pool(name="qk", bufs=2) as qk_pool,
            tc.tile_pool(name="vt", bufs=2) as vt_pool,
            tc.tile_pool(name="vv", bufs=2) as vv_pool,
            tc.tile_pool(name="es", bufs=2) as es_pool,
            tc.tile_pool(name="uo", bufs=2) as uo_pool,
            tc.tile_pool(name="ps_q", bufs=1, space="PSUM") as ps_q,
            tc.tile_pool(name="ps_k", bufs=1, space="PSUM") as ps_k,
            tc.tile_pool(name="ps_v", bufs=1, space="PSUM") as ps_v,
            tc.tile_pool(name="ps_e", bufs=1, space="PSUM") as ps_e,
            tc.tile_pool(name="ps_vu", bufs=1, space="PSUM") as ps_vu,
            tc.tile_pool(name="ps_u", bufs=2, space="PSUM") as ps_u,
        ):
            # ---- constants, loaded once ----
            wq_sb = singles.tile([128, NKP, 2, DK], F8)
            nc.sync.dma_start(out=wq_sb, in_=wq[:, :])
            wk_sb = singles.tile([128, NKP, 2, DK], F8)
            nc.sync.dma_start(out=wk_sb, in_=wk[:, :])
            wv_sb = singles.tile([128, NKP, 2, DK], F8)
            nc.sync.dma_start(out=wv_sb, in_=wv[:, :])
            wr_sb = singles.tile([128, NKP, 2, DK], F8)
            nc.sync.dma_start(out=wr_sb, in_=wr[:, :])
            m1_sb = singles.tile([128, T], BF16)
            nc.sync.dma_start(out=m1_sb, in_=m1[:, :])
            m2_sb = singles.tile([72, 72], BF16)
            nc.sync.dma_start(out=m2_sb, in_=m2[:, :])
            fold_sb = singles.tile([64, DK], BF16)
            nc.sync.dma_start(out=fold_sb, in_=fold[:, :])

            state = {}

            def emit_front(a, b):
                """Folds + scores + exp/mask/V-tiles for batch b of pair a."""
                q_sb, k_sb, vt_sb = state[("qkv", a)]
                vu = state[("vu", a)] if b else ps_vu.tile(
                    [128, VU_COLS], F32, tag="vu")
                e_ps = state[("e", a)] if b else ps_e.tile(
                    [128, 2, T], F32, tag="e_ps")
                if not b:
                    state[("vu", a)] = vu
                    state[("e", a)] = e_ps
                c0 = b * T
                # transpose V^T -> V[t, v] on the PE (identity moving op)
                nc.tensor.matmul(vu[:, VTR0:VTR0 + 64],
                                 vt_sb[:, c0:c0 + 128], fold_sb,
                                 start=True, stop=True)
                nc.tensor.matmul(vu[0:72, VTR1:VTR1 + 64],
                                 vt_sb[:, c0 + 128:c0 + 200], fold_sb,
                                 start=True, stop=True)
                # scores^T: s-tile 1 = all t; s-tile 2 only its valid t>=128
                nc.tensor.matmul(e_ps[:, b, :],
                                 k_sb[:, c0:c0 + 128], q_sb[:, c0:c0 + T],
                                 start=True, stop=True)
                nc.tensor.matmul(vu[0:72, E2_COL:E2_COL + 72],
                                 k_sb[:, c0 + 128:c0 + 200],
                                 q_sb[:, c0 + 128:c0 + 200],
                                 start=True, stop=True)
                # exp on ACT (psum f32 -> sbuf bf16); causal mask on GPSIMD
                e1_sb = es_pool.tile([128, T], BF16, tag="e1_sb")
                nc.scalar.activation(e1_sb, e_ps[:, b, :],
                                     mybir.ActivationFunctionType.Exp,
                                     scale=2.0 ** -15)
                e2_sb = es_pool.tile([72, 72], BF16, tag="e2_sb")
                nc.scalar.activation(e2_sb, vu[0:72, E2_COL:E2_COL + 72],
                                     mybir.ActivationFunctionType.Exp,
                                     scale=2.0 ** -15)
                # V tiles (bf16) + 32-valued ones column
                v0 = vv_pool.tile([128, 65], BF16, tag="v0")
                v1 = vv_pool.tile([72, 65], BF16, tag="v1")
                nc.gpsimd.memset(v0[:, 64:65], 32.0)
                nc.gpsimd.memset(v1[:, 64:65], 32.0)
                nc.vector.tensor_copy(v0[:, 0:64], vu[:, VTR0:VTR0 + 64])
                nc.vector.tensor_copy(v1[:, 0:64], vu[0:72, VTR1:VTR1 + 64])
                nc.gpsimd.tensor_mul(e1_sb, e1_sb, m1_sb)
                nc.gpsimd.tensor_mul(e2_sb, e2_sb, m2_sb)
                return e1_sb, e2_sb, v0, v1

            def emit_back(a, b, front):
                """U matmuls for batch b of pair a: [num | den] into psum."""
                e1_sb, e2_sb, v0, v1 = front
                u_ps = state[("u", a)] if b else ps_u.tile(
                    [100, 2, 130], F32, tag="u_ps")
                if not b:
                    state[("u", a)] = u_ps
                e1p = e1_sb.rearrange("s (t c) -> s t c", c=2)
                e2p = e2_sb.rearrange("s (t c) -> s t c", c=2)
                for ci in range(2):
                    nc.tensor.matmul(u_ps[0:100, b, 65 * ci:65 * ci + 65],
                                     e1p[:, :, ci], v0,
                                     start=True, stop=False)
                    nc.tensor.matmul(u_ps[64:100, b, 65 * ci:65 * ci + 65],
                                     e2p[:, :, ci], v1,
                                     start=False, stop=True,
                                     skip_group_check=True)

            # ---- software-pipelined main loop ----
            # iteration i emits: U(b1) of pair i-2, attention front/U(b0)/
            # front(b1) of pair i-1, projections of pair i, store of pair i-2
            for _rep in range(repeat):
                for i in range(n_pair + 2):
                    a = i - 1
                    if 0 <= i - 2:
                        emit_back(i - 2, 1, state.pop(("f1", i - 2)))
                    if 0 <= a < n_pair:
                        f0 = emit_front(a, 0)
                    if i < n_pair:
                        # input DMA + Q/K projections + their evacuations
                        qrt = qr_pool.tile([128, 2, NKP, 2, 2, T], F8)
                        nc.sync.dma_start(out=qrt, in_=qr[i])
                        q_ps = ps_q.tile([64, 2 * T], F32, tag="q_ps")
                        k_ps = ps_k.tile([64, 2 * T], F32, tag="k_ps")
                        for kp in range(NKP):
                            nc.tensor.matmul(q_ps, wq_sb[:, kp], qrt[:, 0, kp],
                                             start=(kp == 0),
                                             stop=(kp == NKP - 1), perf_mode=DR)
                        for kp in range(NKP):
                            nc.tensor.matmul(k_ps, wk_sb[:, kp], qrt[:, 0, kp],
                                             start=(kp == 0),
                                             stop=(kp == NKP - 1), perf_mode=DR)
                        q_sb = qk_pool.tile([64, 2 * T], BF16, tag="q_sb")
                        nc.vector.tensor_copy(q_sb, q_ps)
                        k_sb = qk_pool.tile([64, 2 * T], BF16, tag="k_sb")
                        nc.scalar.copy(k_sb, k_ps)
                    if 0 <= a < n_pair:
                        emit_back(a, 0, f0)
                        state[("f1", a)] = emit_front(a, 1)
                    if i < n_pair:
                        # V projections: 3 DoubleRow passes, one accumulation
                        # (q8@Wv8 + r8@Wv8 + q8@rho8) -> V^T in [64, 400]
                        v_ps = ps_v.tile([64, 2 * T], F32, tag="v_ps")
                        for kp in range(NKP):
                            nc.tensor.matmul(v_ps, wv_sb[:, kp],
                                             qrt[:, 0, kp],
                                             start=(kp == 0), stop=False,
                                             perf_mode=DR)
                        for kp in range(NKP):
                            nc.tensor.matmul(v_ps, wv_sb[:, kp],
                                             qrt[:, 1, kp],
                                             start=False, stop=False,
                                             perf_mode=DR)
                        for kp in range(NKP):
                            nc.tensor.matmul(v_ps, wr_sb[:, kp],
                                             qrt[:, 0, kp],
                                             start=False,
                                             stop=(kp == NKP - 1),
                                             perf_mode=DR)
                        vt_sb = vt_pool.tile([64, 2 * T], BF16, tag="vt_sb")
                        nc.scalar.copy(vt_sb, v_ps)
                        state[("qkv", i)] = (q_sb, k_sb, vt_sb)
                    if 0 <= i - 2:
                        state.pop(("qkv", i - 2))
                        state.pop(("vu", i - 2))
                        state.pop(("e", i - 2))
                        u_ps = state.pop(("u", i - 2))
                        u_sb = uo_pool.tile([100, 2, 130], F32, tag="u_sb")
                        nc.vector.tensor_copy(u_sb, u_ps)
                        nc.sync.dma_start(
                            out=uo[i - 2].rearrange("t (b x) -> t b x", b=2),
                            in_=u_sb)

    nc.compile()
    return nc


def _f8(x):
    return np.asarray(x, dtype=np.float32).astype(E4M3)


def _host_inputs(q, Wq, Wk, Wv):
    """Quantize + lay out the shared weights and the per-batch q blocks."""
    # weights scaled by 32 -> ~N(0,1), well inside e4m3 range
    def wprep(w8):
        # [1024, 64] -> [128 p, 4 kp, 2 ks, 64]; d = (2kp+ks)*128 + p
        return np.ascontiguousarray(
            w8.reshape(NKP, 2, 128, DK).transpose(2, 0, 1, 3).reshape(128, -1))

    wq32 = 32.0 * np.asarray(Wq, np.float32)
    wk32 = 32.0 * np.asarray(Wk, np.float32)
    wv32 = 32.0 * np.asarray(Wv, np.float32)
    wq8 = _f8(wq32)
    wk8 = _f8(wk32)
    wv8 = _f8(wv32)
    wr8 = _f8(wv32 - wv8.astype(np.float32))

    t_idx = np.arange(T)[None, :]
    m1 = (t_idx >= np.arange(128)[:, None]).astype(NPBF16)
    m2 = (t_idx[:, :72] >= np.arange(72)[:, None]).astype(NPBF16)
    foldm = np.ascontiguousarray(np.eye(DK, dtype=np.float32)).astype(NPBF16)

    q = np.asarray(q, np.float32)
    q8 = q.astype(E4M3)
    r8 = (q - q8.astype(np.float32)).astype(E4M3)
    nb = q.shape[0]

    # [B, T, D] -> [P, p, {q8,r8}, kp, ks, b, t] fp8, 6400B per partition
    def qprep(x):
        return (x.transpose(0, 2, 1)
                .reshape(nb // 2, 2, NKP, 2, 128, T)
                .transpose(0, 4, 2, 3, 1, 5))  # [P, p, kp, ks, b, t]

    qr8 = np.ascontiguousarray(
        np.stack([qprep(q8), qprep(r8)], axis=2)  # [P, p, h, kp, ks, b, t]
        .reshape(nb // 2, 128, -1))
    return qr8, {
        "wq": wprep(wq8), "wk": wprep(wk8), "wv": wprep(wv8), "wr": wprep(wr8),
        "m1": np.ascontiguousarray(m1), "m2": np.ascontiguousarray(m2),
        "fold": foldm,
    }


def _finalize(uo, n_batch):
    """Host-side softmax normalization: [P, 100, 260] -> [n_batch, T, DK]."""
    n_pair = n_batch // 2
    u = uo.reshape(n_pair, 100, 2, 2, 65)        # [P, p, b, c, 65]
    o = u[..., :64] / u[..., 64:65]
    # t = 2p + c, batch = 2P + b
    return np.ascontiguousarray(
        o.transpose(0, 2, 1, 3, 4).reshape(n_batch, T, DK))


_NC_CACHE = {}


def _get_nc(n_batch=B_CORE, repeat=1):
    key = (n_batch, repeat)
    if key not in _NC_CACHE:
        _NC_CACHE[key] = build_nc(n_batch, repeat)
    return _NC_CACHE[key]


def kernel(q, Wq, Wk, Wv):
    q = np.asarray(q, dtype=np.float32)
    qr8, shared = _host_inputs(q, Wq, Wk, Wv)

    nc = _get_nc()
    npair_core = B_CORE // 2
    in_maps = [
        {"qr": np.ascontiguousarray(qr8[c * npair_core:(c + 1) * npair_core]),
         **shared}
        for c in range(N_CORES)
    ]
    res = run_bass_kernel_spmd(nc, in_maps, core_ids=list(range(N_CORES)))
    return np.concatenate(
        [_finalize(np.asarray(r["uo"], np.float32), B_CORE)
         for r in res.results], axis=0)
